# revision 48
# baseline (speedup 1.0000x reference)
"""GQA attention block (RMSNorm + QKV proj + partial RoPE + causal GQA
attention + XSA correction + out proj) on 8 trn2 NeuronCores.

Sharding: 2 batches x 4 KV-groups (each core: 1 batch, 1 kv head, 4 q heads).
Each core computes a partial output (its 4 heads through its wo column slice);
the host sums the 4 partials per batch.

v2 design notes (all-bf16 dataflow):
 - host pre-casts x/wqkv/wo to bf16 and folds w_norm into the projection
   weights; no on-chip dtype casts at all.
 - rope rotate-half is turned into an adjacent-partition swap by permuting
   the first 64 rows of wq/wk (and the cos/sin tables) on the host, so the
   swap is a single DVE stream_shuffle (no sbuf-to-sbuf DMAs).
 - every reciprocal / rsqrt is computed as exp(-ln(x)) on the scalar engine;
   ln+exp live in one activation table set so there is no table thrashing.
 - causal masking uses partial-width score/sum/PV matmuls plus one
   gpsimd affine_select on the 128-wide diagonal block.
 - the PE instruction stream is ordered so QKV(j+1) / outproj(j-1) fill the
   windows where attention waits on rope/XSA, keeping the HAM clock gate
   warm (PE idle >3.4us re-throttles the PE clock 2x).
"""

import sys

for _p in ("/opt/trn_rl_repo", "/root/.axon_site/_ro/trn_rl_repo"):
    if _p not in sys.path:
        sys.path.append(_p)

import numpy as np
import ml_dtypes

import concourse.bass as bass
import concourse.bacc as bacc
import concourse.mybir as mybir
import concourse.tile as tile
from concourse import hw_specs as _hw_specs
from concourse.bass_utils import run_bass_kernel_spmd
from concourse.masks import make_identity

# The activation-table chooser maps Ln -> "natural_log" and Exp ->
# "exp_and_others", so a kernel alternating ln/exp reloads the table RAMs
# (~2.7us) on every switch.  Both functions live together in
# "natural_log_exp_and_others"; restrict them to that set so exactly one
# table load is ever emitted.
_ORIG_GAT = _hw_specs.get_activation_tables


def _gat_combined(arch):
    tabs = _ORIG_GAT(arch)
    keep = "natural_log_exp_and_others"
    if keep in tabs:
        ln = mybir.ActivationFunctionType.Ln
        ex = mybir.ActivationFunctionType.Exp
        for nm, fns in tabs.items():
            if nm != keep:
                fns.discard(ln)
                fns.discard(ex)
    return tabs


_hw_specs.get_activation_tables = _gat_combined
bacc.get_activation_tables = _gat_combined

F32 = mybir.dt.float32
BF16 = mybir.dt.bfloat16

B, T, D = 2, 2048, 2048
NH, NKV, HD = 16, 4, 128
RD = 64                    # rope dims
NH_L = NH // NKV           # 4 q heads per core
EL = (NH_L + 2) * HD       # 768: q0..q3, k, v columns
TC = 512                   # token chunk
NTC = T // TC              # 4
DC = D // 128              # 16 contraction chunks
S128 = float(1.0 / np.sqrt(HD))
EPS = 1e-6

# DVE stream_shuffle mask: swap adjacent partitions within each 32-quadrant
SWAP_MASK = []
for _i in range(16):
    SWAP_MASK += [2 * _i + 1, 2 * _i]

_CACHE = {}


def _build_nc():
    nc = bacc.Bacc("TRN2", target_bir_lowering=False, debug=False)

    xT = nc.declare_dram_parameter("xT", [D, T], BF16, isOutput=False)
    wT = nc.declare_dram_parameter("wqkvT", [D, EL], BF16, isOutput=False)
    woT = nc.declare_dram_parameter("woT", [NH_L * HD, D], BF16, isOutput=False)
    csP = nc.declare_dram_parameter("cs", [128, T], BF16, isOutput=False)
    outp = nc.declare_dram_parameter("out", [T, D], F32, isOutput=True)

    ACT = mybir.ActivationFunctionType

    with tile.TileContext(nc) as tc:
        with (
            nc.allow_low_precision(reason="bf16 dataflow; rel tol 2e-2"),
            tc.tile_pool(name="singles", bufs=1) as sg,
            tc.tile_pool(name="stream", bufs=2) as st,
            tc.tile_pool(name="ps", bufs=1, space="PSUM") as ps,
        ):
            # ---------------- persistent tiles ----------------
            w_sb = sg.tile([128, DC * EL], BF16, tag="w")
            wo_sb = sg.tile([128, 16 * TC], BF16, tag="wo")
            cosP = sg.tile([RD, T], BF16, tag="cosP")
            sinP = sg.tile([RD, T], BF16, tag="sinP")
            ident = sg.tile([128, 128], BF16, tag="ident")
            maskM = sg.tile([128, 128], BF16, tag="maskM")
            ones_cb = sg.tile([128, 1], BF16, tag="ones_cb")
            ones_cf = sg.tile([128, 1], F32, tag="ones_cf")
            eps_t = sg.tile([1, 1], F32, tag="eps_t")
            kh = [sg.tile([128, TC], BF16, tag=f"kh{j}", name=f"kh{j}")
                  for j in range(NTC)]
            vh = [sg.tile([128, TC], BF16, tag=f"vh{j}", name=f"vh{j}")
                  for j in range(NTC)]
            vt = [sg.tile([128, TC], BF16, tag=f"vt{j}", name=f"vt{j}")
                  for j in range(NTC)]
            rvns = [sg.tile([1, TC], F32, tag=f"rvns{j}", name=f"rvns{j}")
                    for j in range(NTC)]

            # x tiles live in a rotating 3-chunk window (SBUF pressure);
            # chunk j's DMAs reuse chunk j-3's buffers once qkv(j-3) is done.
            xtl = {}

            def xsl(j, i):
                return xtl[(j, i)]

            def emit_xload(j):
                js = slice(j * TC, (j + 1) * TC)
                for i in range(DC):
                    xtl[(j, i)] = st.tile([128, TC], BF16, tag="xt",
                                          bufs=3 * DC, name=f"x_{j}_{i}")
                    nc.sync.dma_start(
                        out=xtl[(j, i)], in_=xT[i * 128:(i + 1) * 128, js])

            # ---------------- initial DMAs ----------------
            nc.sync.dma_start(out=cosP, in_=csP[0:RD, :])
            nc.sync.dma_start(out=sinP, in_=csP[RD:128, :])
            # interleave k/v weight columns with x chunk 0 so the i-th
            # pass-B matmul can start as soon as its own tiles land
            for i in range(DC):
                nc.sync.dma_start(
                    out=w_sb[:, i * EL + 4 * HD:(i + 1) * EL],
                    in_=wT[i * 128:(i + 1) * 128, 4 * HD:EL],
                )
                xtl[(0, i)] = st.tile([128, TC], BF16, tag="xt",
                                      bufs=3 * DC, name=f"x_0_{i}")
                # scalar engine is an HWDGE too and idle at start: issuing
                # x0 there doubles the initial DMA issue rate
                nc.scalar.dma_start(
                    out=xtl[(0, i)], in_=xT[i * 128:(i + 1) * 128, 0:TC])
            # q weight columns
            for i in range(DC):
                nc.sync.dma_start(
                    out=w_sb[:, i * EL:i * EL + 4 * HD],
                    in_=wT[i * 128:(i + 1) * 128, 0:4 * HD],
                )
            emit_xload(1)
            emit_xload(2)
            # wo tiles: (h, m) at column (h*4+m)*TC
            for h in range(NH_L):
                for m in range(4):
                    nc.sync.dma_start(
                        out=wo_sb[:, (h * 4 + m) * TC:(h * 4 + m + 1) * TC],
                        in_=woT[h * 128:(h + 1) * 128, m * TC:(m + 1) * TC],
                    )
            # chunk 3's x loads wait on chunk 0's buffers; issue them after wo
            # so the in-order sync queue doesn't hold the wo transfers back.
            emit_xload(3)

            make_identity(nc, ident)
            nc.vector.memset(ones_cb, 1.0)
            nc.vector.memset(ones_cf, 1.0)
            nc.vector.memset(eps_t, EPS)
            # lower-triangle 1/0 mask (keep col >= partition); applying it
            # as a DVE multiply keeps gpsimd's variable-latency
            # affine_select off the attention critical path
            nc.gpsimd.memset(maskM, 1.0)
            nc.gpsimd.affine_select(
                out=maskM, in_=maskM,
                compare_op=mybir.AluOpType.is_ge,
                fill=0.0, base=0,
                pattern=[[1, 128]],
                channel_multiplier=-1,
            )

            # x^2 tiles for the rms-norm sum (created one chunk ahead)
            x2 = {}
            state = {}

            def emit_x2(j):
                # x^2 tiles plus a running DVE accumulation over the 16
                # contraction blocks (two ping-pong accumulators so the
                # serial dependency never stalls the DVE pipe); qkvB then
                # needs a single ones-matmul for the rms sum instead of 16.
                # Chunk 0 instead keeps per-tile PE matmuls (its x tiles
                # trickle in from the initial DMAs).
                acc = st.tile([128, TC], BF16, tag="x2a", bufs=2,
                              name=f"x2a{j}")
                accB = st.tile([128, TC], BF16, tag="x2b", bufs=2,
                               name=f"x2b{j}")
                for i in range(DC):
                    x2[(j, i)] = st.tile([128, TC], BF16, tag="x2",
                                         bufs=4, name=f"x2_{j}_{i}")
                    nc.vector.tensor_mul(x2[(j, i)], xsl(j, i), xsl(j, i))
                    if j == 0:
                        continue
                    a = acc if i % 2 == 0 else accB
                    if i < 2:
                        nc.vector.tensor_copy(a, x2[(j, i)])
                    else:
                        nc.vector.tensor_add(a, a, x2[(j, i)])
                if j > 0:
                    nc.vector.tensor_add(acc, acc, accB)
                state[("x2a", j)] = acc

            def emit_qkvB(j, rs_mode="first"):
                # ---- pass B: rs sum + k, v on PE ----
                sm_rs = ps.tile([1, TC], F32, tag="big", bufs=4,
                                name=f"smrs{j}")
                ps_k = ps.tile([128, TC], F32, tag="big", bufs=4, name=f"psk{j}")
                ps_v = ps.tile([128, TC], F32, tag="big", bufs=4, name=f"psv{j}")
                if rs_mode == "first":
                    # x2a was accumulated on DVE during attn pair A of the
                    # previous chunk, so this never waits
                    nc.tensor.matmul(sm_rs, ones_cb, state[("x2a", j)],
                                     start=True, stop=True)
                for i in range(DC):
                    wof = i * EL
                    nc.tensor.matmul(
                        ps_k, w_sb[:, wof + 4 * HD:wof + 5 * HD], xsl(j, i),
                        start=(i == 0), stop=(i == DC - 1))
                    nc.tensor.matmul(
                        ps_v, w_sb[:, wof + 5 * HD:wof + 6 * HD], xsl(j, i),
                        start=(i == 0), stop=(i == DC - 1))
                    if rs_mode == "pe":
                        # chunk 0: x tiles trickle in from the initial DMAs,
                        # so reduce per-tile on the PE at DMA pace
                        nc.tensor.matmul(
                            sm_rs, ones_cb, x2[(j, i)],
                            start=(i == 0), stop=(i == DC - 1))
                if rs_mode == "last":
                    nc.tensor.matmul(sm_rs, ones_cb, state[("x2a", j)],
                                     start=True, stop=True)
                # rs = exp(-0.5*ln(mean(x^2)+eps))  (scalar engine only)
                lnr = st.tile([1, TC], F32, tag="lnr", bufs=2)
                nc.scalar.activation(lnr, sm_rs, ACT.Ln, scale=1.0 / D,
                                     bias=eps_t)
                rs_t = st.tile([1, TC], BF16, tag="rs_t", bufs=2)
                nc.scalar.activation(rs_t, lnr, ACT.Exp, scale=-0.5)
                rsb = st.tile([128, TC], BF16, tag="rsb", bufs=2)
                nc.gpsimd.partition_broadcast(rsb, rs_t)
                state[("rsb", j)] = (ps_k, ps_v, rsb)

            def emit_qkvA(j):
                js = slice(j * TC, (j + 1) * TC)
                ps_k, ps_v, rsb = state[("rsb", j)]

                def rope(th):
                    t2s = st.tile([RD, TC], BF16, tag="t2s", bufs=2)
                    nc.vector.stream_shuffle(t2s, th[0:RD], SWAP_MASK)
                    nc.vector.tensor_mul(th[0:RD], th[0:RD], cosP[:, js])
                    nc.vector.tensor_mul(t2s, t2s, sinP[:, js])
                    nc.vector.tensor_add(th[0:RD], th[0:RD], t2s)

                qhj = [
                    st.tile([128, TC], BF16, tag="qh", bufs=8,
                            name=f"qh{j}_{h}")
                    for h in range(NH_L)
                ]
                # two 2-head subpasses keep peak PSUM at 4 accumulator banks
                ps_q01 = [ps.tile([128, TC], F32, tag="big", bufs=4,
                                  name=f"psq{j}_{h}") for h in (0, 1)]
                for i in range(DC):
                    wof = i * EL
                    for h in (0, 1):
                        nc.tensor.matmul(
                            ps_q01[h], w_sb[:, wof + h * HD:wof + (h + 1) * HD],
                            xsl(j, i), start=(i == 0), stop=(i == DC - 1))
                # evacuate with rms scale folded in (DVE); runs during sub2
                nc.vector.tensor_mul(kh[j], ps_k, rsb)
                nc.vector.tensor_mul(vh[j], ps_v, rsb)
                nc.vector.tensor_mul(qhj[0], ps_q01[0], rsb)
                nc.vector.tensor_mul(qhj[1], ps_q01[1], rsb)
                rope(kh[j])
                rope(qhj[0])
                rope(qhj[1])
                ps_q23 = [ps.tile([128, TC], F32, tag="big", bufs=4,
                                  name=f"psq{j}_{h + 2}") for h in (0, 1)]
                for i in range(DC):
                    wof = i * EL
                    for h in (0, 1):
                        nc.tensor.matmul(
                            ps_q23[h],
                            w_sb[:, wof + (h + 2) * HD:wof + (h + 3) * HD],
                            xsl(j, i), start=(i == 0), stop=(i == DC - 1))
                vsq = st.tile([128, TC], BF16, tag="vsq", bufs=2,
                              name=f"vsq{j}")
                nc.vector.tensor_mul(vsq, vh[j], vh[j])
                nc.vector.tensor_mul(qhj[2], ps_q23[0], rsb)
                nc.vector.tensor_mul(qhj[3], ps_q23[1], rsb)
                rope(qhj[2])
                rope(qhj[3])
                state[("vsq", j)] = vsq
                state[("qh", j)] = qhj

            def emit_vt(j):
                # v token-major transpose (PE), own phase so the in-order PE
                # queue never waits on vh here
                ps_vt = ps.tile([128, TC], BF16, tag="big", bufs=4,
                                name=f"psvt{j}")
                for kk in range(TC // 128):
                    nc.tensor.transpose(
                        ps_vt[:, kk * 128:(kk + 1) * 128],
                        vh[j][:, kk * 128:(kk + 1) * 128],
                        ident,
                    )
                nc.vector.tensor_copy(vt[j], ps_vt)

            def emit_vns(j):
                vsq = state[("vsq", j)]
                sm_vns = ps.tile([1, TC], F32, tag="big", bufs=4,
                                 name=f"smvns{j}")
                nc.tensor.matmul(sm_vns, ones_cb, vsq, start=True, stop=True)
                lnv = st.tile([1, TC], F32, tag="lnv", bufs=2)
                nc.scalar.activation(lnv, sm_vns, ACT.Ln, scale=1.0,
                                     bias=eps_t)
                nc.scalar.activation(rvns[j], lnv, ACT.Exp, scale=-1.0)

            def emit_attn_pair(j, pair):
                heads = (0, 1) if pair == 0 else (2, 3)
                qhj = state[("qh", j)]
                nkt = 4 * (j + 1)
                ps_pv = {
                    h: ps.tile([128, TC], F32, tag="big", bufs=4,
                               name=f"pspv{j}_{h}")
                    for h in heads
                }
                sm_sum = ps.tile([64, TC], F32, tag="big", bufs=4,
                                 name=f"smsum{j}_{pair}")
                pT = {}

                def tile_geom(kt):
                    cs = 128 * (kt - 4 * j) if kt >= 4 * j else 0
                    return cs, TC - cs

                def emit_sum_pv(kt):
                    # both tiny-LDW sum mms first, then the two PV mms, so
                    # each PV's 128-col LDWEIGHTS prefetches under the
                    # preceding matmul's stream
                    cs, _w = tile_geom(kt)
                    jk, kk = divmod(kt, 4)
                    pt2 = pT[kt]
                    for idx, h in enumerate(heads):
                        nc.tensor.matmul(
                            sm_sum[32 * (h % 2):32 * (h % 2) + 1, cs:TC],
                            ones_cb, pt2[:, idx * TC + cs:(idx + 1) * TC],
                            start=(kt == 0), stop=(kt == nkt - 1))
                    for idx, h in enumerate(heads):
                        nc.tensor.matmul(
                            ps_pv[h][:, cs:TC],
                            vt[jk][:, kk * 128:(kk + 1) * 128],
                            pt2[:, idx * TC + cs:(idx + 1) * TC],
                            start=(kt == 0), stop=(kt == nkt - 1))

                # both heads' scores land in one double-wide PSUM tile so a
                # single exp covers them (halves the scalar engine's per-tile
                # overhead, which gates attention); sum/pv for tile kt-2
                # issue while kt's scores compute so LDWEIGHTS prefetch is
                # never blocked on a pending semaphore.
                for kt in range(nkt):
                    cs, _w = tile_geom(kt)
                    jk, kk = divmod(kt, 4)
                    ps_sc = ps.tile([128, 2 * TC], F32, tag="sc2", bufs=2,
                                    name=f"pssc{j}_{pair}_{kt}")
                    for idx, h in enumerate(heads):
                        nc.tensor.matmul(
                            ps_sc[:, idx * TC + cs:(idx + 1) * TC],
                            kh[jk][:, kk * 128:(kk + 1) * 128],
                            qhj[h][:, cs:TC],
                            start=True, stop=True)
                    pt2 = st.tile([128, 2 * TC], BF16, tag="pT", bufs=4,
                                  name=f"pt{j}_{pair}_{kt}")
                    # the [TC : TC+cs] span holds stale psum when cs>0; it is
                    # exp'd but never read downstream
                    nc.scalar.activation(
                        pt2[:, cs:2 * TC], ps_sc[:, cs:2 * TC], ACT.Exp,
                        scale=S128)
                    if kt >= 4 * j:
                        # zero strictly-above-diagonal entries in the
                        # 128-wide diagonal block (DVE mask multiply)
                        for idx in range(2):
                            sl = pt2[:, idx * TC + cs:idx * TC + cs + 128]
                            nc.vector.tensor_mul(sl, sl, maskM)
                    pT[kt] = pt2
                    if kt > 1:
                        emit_sum_pv(kt - 2)
                if nkt > 1:
                    emit_sum_pv(nkt - 2)
                emit_sum_pv(nkt - 1)
                state[(j, pair)] = (ps_pv, sm_sum)

            def emit_xsa_pre(j, pair):
                heads = (0, 1) if pair == 0 else (2, 3)
                ps_pv, sm_sum = state[(j, pair)]
                pvsb, tu, inv = {}, {}, {}
                for h in heads:
                    pvsb[h] = st.tile([128, TC], BF16, tag="pvsb", bufs=4,
                                      name=f"pvsb{j}_{h}")
                    nc.vector.tensor_copy(pvsb[h], ps_pv[h])
                    lnS = st.tile([1, TC], F32, tag="lnS", bufs=2)
                    nc.scalar.activation(
                        lnS, sm_sum[32 * (h % 2):32 * (h % 2) + 1, :], ACT.Ln)
                    inv[h] = st.tile([1, TC], BF16, tag="inv", bufs=4,
                                     name=f"inv{j}_{h}")
                    nc.scalar.activation(inv[h], lnS, ACT.Exp, scale=-1.0)
                    tu[h] = st.tile([128, TC], BF16, tag="tu", bufs=4,
                                    name=f"tu{j}_{h}")
                    nc.vector.tensor_mul(tu[h], pvsb[h], vh[j])
                state[(j, pair, "pre")] = (pvsb, tu, inv)

            def emit_dots(j, pair):
                heads = (0, 1) if pair == 0 else (2, 3)
                _, tu, _ = state[(j, pair, "pre")]
                sm_dot = ps.tile([64, TC], F32, tag="big", bufs=4,
                                 name=f"smdot{j}_{pair}")
                for h in heads:
                    nc.tensor.matmul(
                        sm_dot[32 * (h % 2):32 * (h % 2) + 1, :],
                        ones_cb, tu[h], start=True, stop=True)
                state[(j, pair, "dot")] = sm_dot

            def emit_xsa_post(j, pair):
                heads = (0, 1) if pair == 0 else (2, 3)
                pvsb, tu, inv = state[(j, pair, "pre")]
                sm_dot = state[(j, pair, "dot")]
                for h in heads:
                    t1 = st.tile([1, TC], F32, tag="t1", bufs=2)
                    nc.vector.tensor_mul(
                        t1, sm_dot[32 * (h % 2):32 * (h % 2) + 1, :], rvns[j])
                    fui = st.tile([1, TC], BF16, tag="fui", bufs=2)
                    nc.vector.tensor_mul(fui, t1, inv[h])
                    invB = st.tile([128, TC], BF16, tag="invB", bufs=2)
                    nc.gpsimd.partition_broadcast(invB, inv[h])
                    fuiB = st.tile([128, TC], BF16, tag="fuiB", bufs=2)
                    nc.gpsimd.partition_broadcast(fuiB, fui)
                    m1 = st.tile([128, TC], BF16, tag="m1", bufs=2)
                    nc.vector.tensor_mul(m1, pvsb[h], invB)
                    m2 = st.tile([128, TC], BF16, tag="m2", bufs=2)
                    nc.vector.tensor_mul(m2, vh[j], fuiB)
                    aot = st.tile([128, TC], BF16, tag="ao", bufs=6,
                                  name=f"ao{j}_{h}")
                    nc.vector.tensor_sub(aot, m1, m2)
                    state[("ao", j, h)] = aot

            def emit_outproj(j, mrange=(0, 1, 2, 3)):
                for m in mrange:
                    ms = slice(m * TC, (m + 1) * TC)
                    for tt in range(4):
                        ps_o = ps.tile([128, TC], F32, tag="big", bufs=4,
                                       name=f"pso{j}_{m}_{tt}")
                        for h in range(NH_L):
                            nc.tensor.matmul(
                                ps_o,
                                state[("ao", j, h)][:, tt * 128:(tt + 1) * 128],
                                wo_sb[:, (h * 4 + m) * TC:(h * 4 + m + 1) * TC],
                                start=(h == 0), stop=(h == NH_L - 1))
                        osb = st.tile([128, TC], F32, tag="osb", bufs=3)
                        # alternate evac engines; with the fused exps the
                        # scalar queue has slack again
                        if tt % 2 == 0:
                            nc.scalar.copy(osb, ps_o)
                        else:
                            nc.vector.tensor_copy(osb, ps_o)
                        nc.sync.dma_start(
                            out=outp[(j * 4 + tt) * 128:(j * 4 + tt + 1) * 128,
                                     ms],
                            in_=osb)

            # ================= the schedule =================
            # PE stream: QKV(0) | attnA(0) attnB(0) dotsA(0) | QKV(1) dotsB(0)
            #            outproj(0) | attnA(1) ... so rope(j+1)/XSA(j) on the
            #            other engines always overlap PE matmul phases.
            # PE stream per j: attnA attnB vns dotsA | qkvB(j+1) dotsB
            # qkvA(j+1) | outproj(j) vt(j+1) | attnA(j+1) ... ; the qkv
            # passes cover the XSA/rope latencies on scalar/DVE/gpsimd.
            emit_x2(0)
            emit_qkvB(0, rs_mode="pe")
            emit_qkvA(0)
            emit_vt(0)
            for j in range(NTC):
                emit_attn_pair(j, 0)
                emit_xsa_pre(j, 0)
                if j + 1 < NTC:
                    # x2a chain runs on DVE during attnB(j)'s PE phase
                    emit_x2(j + 1)
                emit_attn_pair(j, 1)
                emit_xsa_pre(j, 1)
                emit_vns(j)
                emit_dots(j, 0)
                emit_xsa_post(j, 0)
                if j == 3:
                    # second half of outproj(2), deferred here so it covers
                    # the XSA-B latency of the final chunk
                    emit_outproj(2, mrange=(2, 3))
                if j + 1 < NTC:
                    # chunk 1's x2a chain races qkvB(1) (chunk 0's attention
                    # window is short), so only defer the rs matmul there
                    emit_qkvB(j + 1, rs_mode="last" if j == 0 else "first")
                emit_dots(j, 1)
                emit_xsa_post(j, 1)
                if j + 1 < NTC:
                    emit_qkvA(j + 1)
                if j == 2:
                    emit_outproj(j, mrange=(0, 1))
                else:
                    emit_outproj(j)
                if j + 1 < NTC:
                    emit_vt(j + 1)

    nc.compile()
    return nc


def _host_inputs(x, cos, sin, w_norm, wq, wk, wv, wo):
    """Build the 8 per-core input maps (host-side layout prep only)."""
    wn = w_norm.astype(np.float32)
    # rope interleave permutation: pair (i, i+32) -> positions (2i, 2i+1)
    p64 = np.empty(64, np.int64)
    p64[0::2] = np.arange(32)
    p64[1::2] = np.arange(32, 64)
    perm = np.concatenate([p64, np.arange(64, HD)])

    cosT = cos.T.astype(np.float32)        # [64, T], cos[i] == cos[i+32]
    sinT = sin.T.astype(np.float32)
    cosP = cosT[p64]
    sinP = np.empty_like(cosP)
    sinP[0::2] = -sinT[:32]
    sinP[1::2] = sinT[:32]
    cs = np.ascontiguousarray(
        np.concatenate([cosP, sinP], axis=0)).astype(ml_dtypes.bfloat16)

    xTs = [np.ascontiguousarray(x[b].T).astype(ml_dtypes.bfloat16)
           for b in range(B)]
    in_maps = []
    for c in range(8):
        b, g = divmod(c, 4)
        wq_s = (wq[g * NH_L * HD:(g + 1) * NH_L * HD] * wn[None, :]).reshape(
            NH_L, HD, D)[:, perm, :].reshape(NH_L * HD, D)
        wk_s = (wk[g * HD:(g + 1) * HD] * wn[None, :])[perm]
        wv_s = wv[g * HD:(g + 1) * HD] * wn[None, :]
        wqkvT = np.ascontiguousarray(
            np.concatenate([wq_s, wk_s, wv_s], axis=0).T
        ).astype(ml_dtypes.bfloat16)                       # [D, 768]
        woT_s = np.ascontiguousarray(
            wo[:, g * NH_L * HD:(g + 1) * NH_L * HD].T
        ).astype(ml_dtypes.bfloat16)                       # [512, D]
        in_maps.append({
            "xT": xTs[b],
            "wqkvT": wqkvT,
            "woT": woT_s,
            "cs": cs,
        })
    return in_maps


def kernel(x, cos, sin, w_norm, wq, wk, wv, wo, rope_dims=64, use_xsa=1,
           **_unused):
    if "nc" not in _CACHE:
        _CACHE["nc"] = _build_nc()
    nc = _CACHE["nc"]
    in_maps = _host_inputs(
        np.asarray(x), np.asarray(cos), np.asarray(sin), np.asarray(w_norm),
        np.asarray(wq), np.asarray(wk), np.asarray(wv), np.asarray(wo),
    )
    res_obj = run_bass_kernel_spmd(nc, in_maps, list(range(8)))
    _CACHE["last"] = res_obj
    res = res_obj.results
    out = np.zeros((B, T, D), dtype=np.float32)
    for c in range(8):
        b = c // 4
        out[b] += np.asarray(res[c]["out"], dtype=np.float32)
    return out


# revision 50
# speedup vs baseline: 1.1420x; 1.1420x over previous
"""GQA attention block (RMSNorm + QKV proj + partial RoPE + causal GQA
attention + XSA correction + out proj) on 8 trn2 NeuronCores.

Sharding: 2 batches x 4 KV-groups (each core: 1 batch, 1 kv head, 4 q heads).
Each core computes a partial output (its 4 heads through its wo column slice);
the host sums the 4 partials per batch.

v2 design notes (all-bf16 dataflow):
 - host pre-casts x/wqkv/wo to bf16 and folds w_norm into the projection
   weights; no on-chip dtype casts at all.
 - rope rotate-half is turned into an adjacent-partition swap by permuting
   the first 64 rows of wq/wk (and the cos/sin tables) on the host, so the
   swap is a single DVE stream_shuffle (no sbuf-to-sbuf DMAs).
 - every reciprocal / rsqrt is computed as exp(-ln(x)) on the scalar engine;
   ln+exp live in one activation table set so there is no table thrashing.
 - causal masking uses partial-width score/sum/PV matmuls plus one
   gpsimd affine_select on the 128-wide diagonal block.
 - the PE instruction stream is ordered so QKV(j+1) / outproj(j-1) fill the
   windows where attention waits on rope/XSA, keeping the HAM clock gate
   warm (PE idle >3.4us re-throttles the PE clock 2x).
"""

import sys

for _p in ("/opt/trn_rl_repo", "/root/.axon_site/_ro/trn_rl_repo"):
    if _p not in sys.path:
        sys.path.append(_p)

import numpy as np
import ml_dtypes

import concourse.bass as bass
import concourse.bacc as bacc
import concourse.mybir as mybir
import concourse.tile as tile
from concourse import hw_specs as _hw_specs
from concourse.bass_utils import run_bass_kernel_spmd
from concourse.masks import make_identity

# The activation-table chooser maps Ln -> "natural_log" and Exp ->
# "exp_and_others", so a kernel alternating ln/exp reloads the table RAMs
# (~2.7us) on every switch.  Both functions live together in
# "natural_log_exp_and_others"; restrict them to that set so exactly one
# table load is ever emitted.
_ORIG_GAT = _hw_specs.get_activation_tables


def _gat_combined(arch):
    tabs = _ORIG_GAT(arch)
    keep = "natural_log_exp_and_others"
    if keep in tabs:
        ln = mybir.ActivationFunctionType.Ln
        ex = mybir.ActivationFunctionType.Exp
        for nm, fns in tabs.items():
            if nm != keep:
                fns.discard(ln)
                fns.discard(ex)
    return tabs


_hw_specs.get_activation_tables = _gat_combined
bacc.get_activation_tables = _gat_combined

F32 = mybir.dt.float32
BF16 = mybir.dt.bfloat16

B, T, D = 2, 2048, 2048
NH, NKV, HD = 16, 4, 128
RD = 64                    # rope dims
NH_L = NH // NKV           # 4 q heads per core
EL = (NH_L + 2) * HD       # 768: q0..q3, k, v columns
TC = 512                   # token chunk
NTC = T // TC              # 4
DC = D // 128              # 16 contraction chunks
S128 = float(1.0 / np.sqrt(HD))
EPS = 1e-6

# DVE stream_shuffle mask: swap adjacent partitions within each 32-quadrant
SWAP_MASK = []
for _i in range(16):
    SWAP_MASK += [2 * _i + 1, 2 * _i]

_CACHE = {}


def _build_nc():
    nc = bacc.Bacc("TRN2", target_bir_lowering=False, debug=False)

    xT = nc.declare_dram_parameter("xT", [D, T], BF16, isOutput=False)
    wT = nc.declare_dram_parameter("wqkvT", [D, EL], BF16, isOutput=False)
    woT = nc.declare_dram_parameter("woT", [NH_L * HD, D], BF16, isOutput=False)
    csP = nc.declare_dram_parameter("cs", [128, T], BF16, isOutput=False)
    outp = nc.declare_dram_parameter("out", [T, D], F32, isOutput=True)

    ACT = mybir.ActivationFunctionType

    with tile.TileContext(nc) as tc:
        with (
            nc.allow_low_precision(reason="bf16 dataflow; rel tol 2e-2"),
            tc.tile_pool(name="singles", bufs=1) as sg,
            tc.tile_pool(name="stream", bufs=2) as st,
            tc.tile_pool(name="ps", bufs=1, space="PSUM") as ps,
        ):
            # ---------------- persistent tiles ----------------
            w_sb = sg.tile([128, DC * EL], BF16, tag="w")
            wo_sb = sg.tile([128, 16 * TC], BF16, tag="wo")
            cosP = sg.tile([RD, T], BF16, tag="cosP")
            sinP = sg.tile([RD, T], BF16, tag="sinP")
            ident = sg.tile([128, 128], BF16, tag="ident")
            maskM = sg.tile([128, 128], BF16, tag="maskM")
            ones_cb = sg.tile([128, 1], BF16, tag="ones_cb")
            ones_cf = sg.tile([128, 1], F32, tag="ones_cf")
            eps_t = sg.tile([1, 1], F32, tag="eps_t")
            kh = [sg.tile([128, TC], BF16, tag=f"kh{j}", name=f"kh{j}")
                  for j in range(NTC)]
            vh = [sg.tile([128, TC], BF16, tag=f"vh{j}", name=f"vh{j}")
                  for j in range(NTC)]
            vt = [sg.tile([128, TC], BF16, tag=f"vt{j}", name=f"vt{j}")
                  for j in range(NTC)]
            rvns = [sg.tile([1, TC], F32, tag=f"rvns{j}", name=f"rvns{j}")
                    for j in range(NTC)]

            # x tiles live in a rotating 3-chunk window (SBUF pressure);
            # chunk j's DMAs reuse chunk j-3's buffers once qkv(j-3) is done.
            xtl = {}

            def xsl(j, i):
                return xtl[(j, i)]

            def emit_xload(j):
                js = slice(j * TC, (j + 1) * TC)
                for i in range(DC):
                    xtl[(j, i)] = st.tile([128, TC], BF16, tag="xt",
                                          bufs=3 * DC, name=f"x_{j}_{i}")
                    nc.sync.dma_start(
                        out=xtl[(j, i)], in_=xT[i * 128:(i + 1) * 128, js])

            # ---------------- initial DMAs ----------------
            nc.sync.dma_start(out=cosP, in_=csP[0:RD, :])
            nc.sync.dma_start(out=sinP, in_=csP[RD:128, :])
            # interleave k/v weight columns with x chunk 0 so the i-th
            # pass-B matmul can start as soon as its own tiles land
            for i in range(DC):
                nc.sync.dma_start(
                    out=w_sb[:, i * EL + 4 * HD:(i + 1) * EL],
                    in_=wT[i * 128:(i + 1) * 128, 4 * HD:EL],
                )
                xtl[(0, i)] = st.tile([128, TC], BF16, tag="xt",
                                      bufs=3 * DC, name=f"x_0_{i}")
                # scalar engine is an HWDGE too and idle at start: issuing
                # x0 there doubles the initial DMA issue rate
                nc.scalar.dma_start(
                    out=xtl[(0, i)], in_=xT[i * 128:(i + 1) * 128, 0:TC])
            # q weight columns
            for i in range(DC):
                nc.sync.dma_start(
                    out=w_sb[:, i * EL:i * EL + 4 * HD],
                    in_=wT[i * 128:(i + 1) * 128, 0:4 * HD],
                )
            emit_xload(1)
            emit_xload(2)
            # wo tiles: (h, m) at column (h*4+m)*TC
            for h in range(NH_L):
                for m in range(4):
                    nc.sync.dma_start(
                        out=wo_sb[:, (h * 4 + m) * TC:(h * 4 + m + 1) * TC],
                        in_=woT[h * 128:(h + 1) * 128, m * TC:(m + 1) * TC],
                    )
            # chunk 3's x loads wait on chunk 0's buffers; issue them after wo
            # so the in-order sync queue doesn't hold the wo transfers back.
            emit_xload(3)

            make_identity(nc, ident)
            nc.vector.memset(ones_cb, 1.0)
            nc.vector.memset(ones_cf, 1.0)
            nc.vector.memset(eps_t, EPS)
            # lower-triangle 1/0 mask (keep col >= partition); applying it
            # as a DVE multiply keeps gpsimd's variable-latency
            # affine_select off the attention critical path
            nc.gpsimd.memset(maskM, 1.0)
            nc.gpsimd.affine_select(
                out=maskM, in_=maskM,
                compare_op=mybir.AluOpType.is_ge,
                fill=0.0, base=0,
                pattern=[[1, 128]],
                channel_multiplier=-1,
            )

            # x^2 tiles for the rms-norm sum (created one chunk ahead)
            x2 = {}
            state = {}

            def emit_x2(j):
                # x^2 tiles plus a running DVE accumulation over the 16
                # contraction blocks (two ping-pong accumulators so the
                # serial dependency never stalls the DVE pipe); qkvB then
                # needs a single ones-matmul for the rms sum instead of 16.
                # Chunk 0 instead keeps per-tile PE matmuls (its x tiles
                # trickle in from the initial DMAs).
                acc = st.tile([128, TC], BF16, tag="x2a", bufs=2,
                              name=f"x2a{j}")
                accB = st.tile([128, TC], BF16, tag="x2b", bufs=2,
                               name=f"x2b{j}")
                for i in range(DC):
                    x2[(j, i)] = st.tile([128, TC], BF16, tag="x2",
                                         bufs=4, name=f"x2_{j}_{i}")
                    nc.vector.tensor_mul(x2[(j, i)], xsl(j, i), xsl(j, i))
                    if j == 0:
                        continue
                    a = acc if i % 2 == 0 else accB
                    if i < 2:
                        nc.vector.tensor_copy(a, x2[(j, i)])
                    else:
                        nc.vector.tensor_add(a, a, x2[(j, i)])
                if j > 0:
                    nc.vector.tensor_add(acc, acc, accB)
                state[("x2a", j)] = acc

            def emit_qkvB(j, rs_mode="first"):
                # ---- pass B: rs sum + k, v on PE ----
                sm_rs = ps.tile([1, TC], F32, tag="big", bufs=4,
                                name=f"smrs{j}")
                ps_k = ps.tile([128, TC], F32, tag="big", bufs=4, name=f"psk{j}")
                ps_v = ps.tile([128, TC], F32, tag="big", bufs=4, name=f"psv{j}")
                if rs_mode == "first":
                    # x2a was accumulated on DVE during attn pair A of the
                    # previous chunk, so this never waits
                    nc.tensor.matmul(sm_rs, ones_cb, state[("x2a", j)],
                                     start=True, stop=True)
                for i in range(DC):
                    wof = i * EL
                    nc.tensor.matmul(
                        ps_k, w_sb[:, wof + 4 * HD:wof + 5 * HD], xsl(j, i),
                        start=(i == 0), stop=(i == DC - 1))
                    nc.tensor.matmul(
                        ps_v, w_sb[:, wof + 5 * HD:wof + 6 * HD], xsl(j, i),
                        start=(i == 0), stop=(i == DC - 1))
                    if rs_mode == "pe":
                        # chunk 0: x tiles trickle in from the initial DMAs,
                        # so reduce per-tile on the PE at DMA pace
                        nc.tensor.matmul(
                            sm_rs, ones_cb, x2[(j, i)],
                            start=(i == 0), stop=(i == DC - 1))
                if rs_mode == "last":
                    nc.tensor.matmul(sm_rs, ones_cb, state[("x2a", j)],
                                     start=True, stop=True)
                # rs = exp(-0.5*ln(mean(x^2)+eps))  (scalar engine only)
                lnr = st.tile([1, TC], F32, tag="lnr", bufs=2)
                nc.scalar.activation(lnr, sm_rs, ACT.Ln, scale=1.0 / D,
                                     bias=eps_t)
                rs_t = st.tile([1, TC], BF16, tag="rs_t", bufs=2)
                nc.scalar.activation(rs_t, lnr, ACT.Exp, scale=-0.5)
                rsb = st.tile([128, TC], BF16, tag="rsb", bufs=2)
                nc.gpsimd.partition_broadcast(rsb, rs_t)
                state[("rsb", j)] = (ps_k, ps_v, rsb)

            def emit_qkvA(j):
                js = slice(j * TC, (j + 1) * TC)
                ps_k, ps_v, rsb = state[("rsb", j)]

                def rope(th):
                    t2s = st.tile([RD, TC], BF16, tag="t2s", bufs=2)
                    nc.vector.stream_shuffle(t2s, th[0:RD], SWAP_MASK)
                    nc.vector.tensor_mul(th[0:RD], th[0:RD], cosP[:, js])
                    nc.vector.tensor_mul(t2s, t2s, sinP[:, js])
                    nc.vector.tensor_add(th[0:RD], th[0:RD], t2s)

                qhj = [
                    st.tile([128, TC], BF16, tag="qh", bufs=8,
                            name=f"qh{j}_{h}")
                    for h in range(NH_L)
                ]
                # two 2-head subpasses keep peak PSUM at 4 accumulator banks
                ps_q01 = [ps.tile([128, TC], F32, tag="big", bufs=4,
                                  name=f"psq{j}_{h}") for h in (0, 1)]
                for i in range(DC):
                    wof = i * EL
                    for h in (0, 1):
                        nc.tensor.matmul(
                            ps_q01[h], w_sb[:, wof + h * HD:wof + (h + 1) * HD],
                            xsl(j, i), start=(i == 0), stop=(i == DC - 1))
                # evacuate with rms scale folded in (DVE); runs during sub2
                nc.vector.tensor_mul(kh[j], ps_k, rsb)
                nc.vector.tensor_mul(vh[j], ps_v, rsb)
                nc.vector.tensor_mul(qhj[0], ps_q01[0], rsb)
                nc.vector.tensor_mul(qhj[1], ps_q01[1], rsb)
                rope(kh[j])
                rope(qhj[0])
                rope(qhj[1])
                ps_q23 = [ps.tile([128, TC], F32, tag="big", bufs=4,
                                  name=f"psq{j}_{h + 2}") for h in (0, 1)]
                for i in range(DC):
                    wof = i * EL
                    for h in (0, 1):
                        nc.tensor.matmul(
                            ps_q23[h],
                            w_sb[:, wof + (h + 2) * HD:wof + (h + 3) * HD],
                            xsl(j, i), start=(i == 0), stop=(i == DC - 1))
                vsq = st.tile([128, TC], BF16, tag="vsq", bufs=2,
                              name=f"vsq{j}")
                nc.vector.tensor_mul(vsq, vh[j], vh[j])
                nc.vector.tensor_mul(qhj[2], ps_q23[0], rsb)
                nc.vector.tensor_mul(qhj[3], ps_q23[1], rsb)
                rope(qhj[2])
                rope(qhj[3])
                state[("vsq", j)] = vsq
                state[("qh", j)] = qhj

            def emit_vt(j):
                # v token-major transpose (PE), own phase so the in-order PE
                # queue never waits on vh here
                ps_vt = ps.tile([128, TC], BF16, tag="big", bufs=4,
                                name=f"psvt{j}")
                for kk in range(TC // 128):
                    nc.tensor.transpose(
                        ps_vt[:, kk * 128:(kk + 1) * 128],
                        vh[j][:, kk * 128:(kk + 1) * 128],
                        ident,
                    )
                nc.vector.tensor_copy(vt[j], ps_vt)

            def emit_vns(j):
                vsq = state[("vsq", j)]
                sm_vns = ps.tile([1, TC], F32, tag="big", bufs=4,
                                 name=f"smvns{j}")
                nc.tensor.matmul(sm_vns, ones_cb, vsq, start=True, stop=True)
                lnv = st.tile([1, TC], F32, tag="lnv", bufs=2)
                nc.scalar.activation(lnv, sm_vns, ACT.Ln, scale=1.0,
                                     bias=eps_t)
                nc.scalar.activation(rvns[j], lnv, ACT.Exp, scale=-1.0)

            def emit_attn_pair(j, pair):
                heads = (0, 1) if pair == 0 else (2, 3)
                qhj = state[("qh", j)]
                nkt = 4 * (j + 1)
                ps_pv = {
                    h: ps.tile([128, TC], F32, tag="big", bufs=4,
                               name=f"pspv{j}_{h}")
                    for h in heads
                }
                sm_sum = ps.tile([64, TC], F32, tag="big", bufs=4,
                                 name=f"smsum{j}_{pair}")
                pT = {}

                def tile_geom(kt):
                    cs = 128 * (kt - 4 * j) if kt >= 4 * j else 0
                    return cs, TC - cs

                def emit_sum_pv(kt):
                    # both tiny-LDW sum mms first, then the two PV mms, so
                    # each PV's 128-col LDWEIGHTS prefetches under the
                    # preceding matmul's stream
                    cs, _w = tile_geom(kt)
                    jk, kk = divmod(kt, 4)
                    pt2 = pT[kt]
                    for idx, h in enumerate(heads):
                        nc.tensor.matmul(
                            sm_sum[32 * (h % 2):32 * (h % 2) + 1, cs:TC],
                            ones_cb, pt2[:, idx * TC + cs:(idx + 1) * TC],
                            start=(kt == 0), stop=(kt == nkt - 1))
                    for idx, h in enumerate(heads):
                        nc.tensor.matmul(
                            ps_pv[h][:, cs:TC],
                            vt[jk][:, kk * 128:(kk + 1) * 128],
                            pt2[:, idx * TC + cs:(idx + 1) * TC],
                            start=(kt == 0), stop=(kt == nkt - 1))

                # both heads' scores land in one double-wide PSUM tile so a
                # single exp covers them (halves the scalar engine's per-tile
                # overhead, which gates attention); sum/pv for tile kt-2
                # issue while kt's scores compute so LDWEIGHTS prefetch is
                # never blocked on a pending semaphore.
                for kt in range(nkt):
                    cs, _w = tile_geom(kt)
                    jk, kk = divmod(kt, 4)
                    ps_sc = ps.tile([128, 2 * TC], F32, tag="sc2", bufs=2,
                                    name=f"pssc{j}_{pair}_{kt}")
                    for idx, h in enumerate(heads):
                        nc.tensor.matmul(
                            ps_sc[:, idx * TC + cs:(idx + 1) * TC],
                            kh[jk][:, kk * 128:(kk + 1) * 128],
                            qhj[h][:, cs:TC],
                            start=True, stop=True)
                    pt2 = st.tile([128, 2 * TC], BF16, tag="pT", bufs=4,
                                  name=f"pt{j}_{pair}_{kt}")
                    # the [TC : TC+cs] span holds stale psum when cs>0; it is
                    # exp'd but never read downstream
                    nc.scalar.activation(
                        pt2[:, cs:2 * TC], ps_sc[:, cs:2 * TC], ACT.Exp,
                        scale=S128)
                    if kt >= 4 * j:
                        # zero strictly-above-diagonal entries in the
                        # 128-wide diagonal block: keep col >= partition.
                        # gpsimd affine_select: on the DVE the wait-on-exp
                        # would serialize the whole attention pipeline.
                        for idx in range(2):
                            nc.gpsimd.affine_select(
                                out=pt2[:, idx * TC + cs:idx * TC + cs + 128],
                                in_=pt2[:, idx * TC + cs:idx * TC + cs + 128],
                                compare_op=mybir.AluOpType.is_ge,
                                fill=0.0, base=0,
                                pattern=[[1, 128]],
                                channel_multiplier=-1,
                            )
                    pT[kt] = pt2
                    if kt > 1:
                        emit_sum_pv(kt - 2)
                if nkt > 1:
                    emit_sum_pv(nkt - 2)
                emit_sum_pv(nkt - 1)
                state[(j, pair)] = (ps_pv, sm_sum)

            def emit_xsa_pre(j, pair):
                heads = (0, 1) if pair == 0 else (2, 3)
                ps_pv, sm_sum = state[(j, pair)]
                pvsb, tu, inv = {}, {}, {}
                for h in heads:
                    pvsb[h] = st.tile([128, TC], BF16, tag="pvsb", bufs=4,
                                      name=f"pvsb{j}_{h}")
                    nc.vector.tensor_copy(pvsb[h], ps_pv[h])
                    lnS = st.tile([1, TC], F32, tag="lnS", bufs=2)
                    nc.scalar.activation(
                        lnS, sm_sum[32 * (h % 2):32 * (h % 2) + 1, :], ACT.Ln)
                    inv[h] = st.tile([1, TC], BF16, tag="inv", bufs=4,
                                     name=f"inv{j}_{h}")
                    nc.scalar.activation(inv[h], lnS, ACT.Exp, scale=-1.0)
                    tu[h] = st.tile([128, TC], BF16, tag="tu", bufs=4,
                                    name=f"tu{j}_{h}")
                    nc.vector.tensor_mul(tu[h], pvsb[h], vh[j])
                state[(j, pair, "pre")] = (pvsb, tu, inv)

            def emit_dots(j, pair):
                heads = (0, 1) if pair == 0 else (2, 3)
                _, tu, _ = state[(j, pair, "pre")]
                sm_dot = ps.tile([64, TC], F32, tag="big", bufs=4,
                                 name=f"smdot{j}_{pair}")
                for h in heads:
                    nc.tensor.matmul(
                        sm_dot[32 * (h % 2):32 * (h % 2) + 1, :],
                        ones_cb, tu[h], start=True, stop=True)
                state[(j, pair, "dot")] = sm_dot

            def emit_xsa_post(j, pair):
                heads = (0, 1) if pair == 0 else (2, 3)
                pvsb, tu, inv = state[(j, pair, "pre")]
                sm_dot = state[(j, pair, "dot")]
                for h in heads:
                    t1 = st.tile([1, TC], F32, tag="t1", bufs=2)
                    nc.vector.tensor_mul(
                        t1, sm_dot[32 * (h % 2):32 * (h % 2) + 1, :], rvns[j])
                    fui = st.tile([1, TC], BF16, tag="fui", bufs=2)
                    nc.vector.tensor_mul(fui, t1, inv[h])
                    invB = st.tile([128, TC], BF16, tag="invB", bufs=2)
                    nc.gpsimd.partition_broadcast(invB, inv[h])
                    fuiB = st.tile([128, TC], BF16, tag="fuiB", bufs=2)
                    nc.gpsimd.partition_broadcast(fuiB, fui)
                    m1 = st.tile([128, TC], BF16, tag="m1", bufs=2)
                    nc.vector.tensor_mul(m1, pvsb[h], invB)
                    m2 = st.tile([128, TC], BF16, tag="m2", bufs=2)
                    nc.vector.tensor_mul(m2, vh[j], fuiB)
                    aot = st.tile([128, TC], BF16, tag="ao", bufs=6,
                                  name=f"ao{j}_{h}")
                    nc.vector.tensor_sub(aot, m1, m2)
                    state[("ao", j, h)] = aot

            def emit_outproj(j, mrange=(0, 1, 2, 3)):
                for m in mrange:
                    ms = slice(m * TC, (m + 1) * TC)
                    for tt in range(4):
                        ps_o = ps.tile([128, TC], F32, tag="big", bufs=4,
                                       name=f"pso{j}_{m}_{tt}")
                        for h in range(NH_L):
                            nc.tensor.matmul(
                                ps_o,
                                state[("ao", j, h)][:, tt * 128:(tt + 1) * 128],
                                wo_sb[:, (h * 4 + m) * TC:(h * 4 + m + 1) * TC],
                                start=(h == 0), stop=(h == NH_L - 1))
                        osb = st.tile([128, TC], F32, tag="osb", bufs=3)
                        # DVE evac: the scalar queue must stay clear for the
                        # next attention phase's exps
                        nc.vector.tensor_copy(osb, ps_o)
                        nc.sync.dma_start(
                            out=outp[(j * 4 + tt) * 128:(j * 4 + tt + 1) * 128,
                                     ms],
                            in_=osb)

            # ================= the schedule =================
            # PE stream: QKV(0) | attnA(0) attnB(0) dotsA(0) | QKV(1) dotsB(0)
            #            outproj(0) | attnA(1) ... so rope(j+1)/XSA(j) on the
            #            other engines always overlap PE matmul phases.
            # PE stream per j: attnA attnB vns dotsA | qkvB(j+1) dotsB
            # qkvA(j+1) | outproj(j) vt(j+1) | attnA(j+1) ... ; the qkv
            # passes cover the XSA/rope latencies on scalar/DVE/gpsimd.
            emit_x2(0)
            emit_qkvB(0, rs_mode="pe")
            emit_qkvA(0)
            emit_vt(0)
            for j in range(NTC):
                emit_attn_pair(j, 0)
                emit_xsa_pre(j, 0)
                if j + 1 < NTC:
                    # x2a chain runs on DVE during attnB(j)'s PE phase
                    emit_x2(j + 1)
                emit_attn_pair(j, 1)
                emit_xsa_pre(j, 1)
                emit_vns(j)
                emit_dots(j, 0)
                emit_xsa_post(j, 0)
                if j == 3:
                    # second half of outproj(2), deferred here so it covers
                    # the XSA-B latency of the final chunk
                    emit_outproj(2, mrange=(2, 3))
                if j + 1 < NTC:
                    # chunk 1's x2a chain races qkvB(1) (chunk 0's attention
                    # window is short), so only defer the rs matmul there
                    emit_qkvB(j + 1, rs_mode="last" if j == 0 else "first")
                emit_dots(j, 1)
                emit_xsa_post(j, 1)
                if j + 1 < NTC:
                    emit_qkvA(j + 1)
                if j == 2:
                    emit_outproj(j, mrange=(0, 1))
                else:
                    emit_outproj(j)
                if j + 1 < NTC:
                    emit_vt(j + 1)

    nc.compile()
    return nc


def _host_inputs(x, cos, sin, w_norm, wq, wk, wv, wo):
    """Build the 8 per-core input maps (host-side layout prep only)."""
    wn = w_norm.astype(np.float32)
    # rope interleave permutation: pair (i, i+32) -> positions (2i, 2i+1)
    p64 = np.empty(64, np.int64)
    p64[0::2] = np.arange(32)
    p64[1::2] = np.arange(32, 64)
    perm = np.concatenate([p64, np.arange(64, HD)])

    cosT = cos.T.astype(np.float32)        # [64, T], cos[i] == cos[i+32]
    sinT = sin.T.astype(np.float32)
    cosP = cosT[p64]
    sinP = np.empty_like(cosP)
    sinP[0::2] = -sinT[:32]
    sinP[1::2] = sinT[:32]
    cs = np.ascontiguousarray(
        np.concatenate([cosP, sinP], axis=0)).astype(ml_dtypes.bfloat16)

    xTs = [np.ascontiguousarray(x[b].T).astype(ml_dtypes.bfloat16)
           for b in range(B)]
    in_maps = []
    for c in range(8):
        b, g = divmod(c, 4)
        wq_s = (wq[g * NH_L * HD:(g + 1) * NH_L * HD] * wn[None, :]).reshape(
            NH_L, HD, D)[:, perm, :].reshape(NH_L * HD, D)
        wk_s = (wk[g * HD:(g + 1) * HD] * wn[None, :])[perm]
        wv_s = wv[g * HD:(g + 1) * HD] * wn[None, :]
        wqkvT = np.ascontiguousarray(
            np.concatenate([wq_s, wk_s, wv_s], axis=0).T
        ).astype(ml_dtypes.bfloat16)                       # [D, 768]
        woT_s = np.ascontiguousarray(
            wo[:, g * NH_L * HD:(g + 1) * NH_L * HD].T
        ).astype(ml_dtypes.bfloat16)                       # [512, D]
        in_maps.append({
            "xT": xTs[b],
            "wqkvT": wqkvT,
            "woT": woT_s,
            "cs": cs,
        })
    return in_maps


def kernel(x, cos, sin, w_norm, wq, wk, wv, wo, rope_dims=64, use_xsa=1,
           **_unused):
    if "nc" not in _CACHE:
        _CACHE["nc"] = _build_nc()
    nc = _CACHE["nc"]
    in_maps = _host_inputs(
        np.asarray(x), np.asarray(cos), np.asarray(sin), np.asarray(w_norm),
        np.asarray(wq), np.asarray(wk), np.asarray(wv), np.asarray(wo),
    )
    res_obj = run_bass_kernel_spmd(nc, in_maps, list(range(8)))
    _CACHE["last"] = res_obj
    res = res_obj.results
    out = np.zeros((B, T, D), dtype=np.float32)
    for c in range(8):
        b = c // 4
        out[b] += np.asarray(res[c]["out"], dtype=np.float32)
    return out


# revision 54
# speedup vs baseline: 1.1520x; 1.0088x over previous
"""GQA attention block (RMSNorm + QKV proj + partial RoPE + causal GQA
attention + XSA correction + out proj) on 8 trn2 NeuronCores.

Sharding: 2 batches x 4 KV-groups (each core: 1 batch, 1 kv head, 4 q heads).
Each core computes a partial output (its 4 heads through its wo column slice);
the host sums the 4 partials per batch.

v2 design notes (all-bf16 dataflow):
 - host pre-casts x/wqkv/wo to bf16 and folds w_norm into the projection
   weights; no on-chip dtype casts at all.
 - rope rotate-half is turned into an adjacent-partition swap by permuting
   the first 64 rows of wq/wk (and the cos/sin tables) on the host, so the
   swap is a single DVE stream_shuffle (no sbuf-to-sbuf DMAs).
 - every reciprocal / rsqrt is computed as exp(-ln(x)) on the scalar engine;
   ln+exp live in one activation table set so there is no table thrashing.
 - causal masking uses partial-width score/sum/PV matmuls plus one
   gpsimd affine_select on the 128-wide diagonal block.
 - the PE instruction stream is ordered so QKV(j+1) / outproj(j-1) fill the
   windows where attention waits on rope/XSA, keeping the HAM clock gate
   warm (PE idle >3.4us re-throttles the PE clock 2x).
"""

import sys

for _p in ("/opt/trn_rl_repo", "/root/.axon_site/_ro/trn_rl_repo"):
    if _p not in sys.path:
        sys.path.append(_p)

import numpy as np
import ml_dtypes

import concourse.bass as bass
import concourse.bacc as bacc
import concourse.mybir as mybir
import concourse.tile as tile
from concourse import hw_specs as _hw_specs
from concourse.bass_utils import run_bass_kernel_spmd
from concourse.masks import make_identity

# The activation-table chooser maps Ln -> "natural_log" and Exp ->
# "exp_and_others", so a kernel alternating ln/exp reloads the table RAMs
# (~2.7us) on every switch.  Both functions live together in
# "natural_log_exp_and_others"; restrict them to that set so exactly one
# table load is ever emitted.
_ORIG_GAT = _hw_specs.get_activation_tables


def _gat_combined(arch):
    tabs = _ORIG_GAT(arch)
    keep = "natural_log_exp_and_others"
    if keep in tabs:
        ln = mybir.ActivationFunctionType.Ln
        ex = mybir.ActivationFunctionType.Exp
        for nm, fns in tabs.items():
            if nm != keep:
                fns.discard(ln)
                fns.discard(ex)
    return tabs


_hw_specs.get_activation_tables = _gat_combined
bacc.get_activation_tables = _gat_combined

F32 = mybir.dt.float32
BF16 = mybir.dt.bfloat16

B, T, D = 2, 2048, 2048
NH, NKV, HD = 16, 4, 128
RD = 64                    # rope dims
NH_L = NH // NKV           # 4 q heads per core
EL = (NH_L + 2) * HD       # 768: q0..q3, k, v columns
TC = 512                   # token chunk
NTC = T // TC              # 4
DC = D // 128              # 16 contraction chunks
S128 = float(1.0 / np.sqrt(HD))
EPS = 1e-6

# DVE stream_shuffle mask: swap adjacent partitions within each 32-quadrant
SWAP_MASK = []
for _i in range(16):
    SWAP_MASK += [2 * _i + 1, 2 * _i]

_CACHE = {}


def _build_nc():
    nc = bacc.Bacc("TRN2", target_bir_lowering=False, debug=False)

    xT = nc.declare_dram_parameter("xT", [D, T], BF16, isOutput=False)
    wT = nc.declare_dram_parameter("wqkvT", [D, EL], BF16, isOutput=False)
    woT = nc.declare_dram_parameter("woT", [NH_L * HD, D], BF16, isOutput=False)
    csP = nc.declare_dram_parameter("cs", [128, T], BF16, isOutput=False)
    outp = nc.declare_dram_parameter("out", [T, D], F32, isOutput=True)

    ACT = mybir.ActivationFunctionType

    with tile.TileContext(nc) as tc:
        with (
            nc.allow_low_precision(reason="bf16 dataflow; rel tol 2e-2"),
            tc.tile_pool(name="singles", bufs=1) as sg,
            tc.tile_pool(name="stream", bufs=2) as st,
            tc.tile_pool(name="ps", bufs=1, space="PSUM") as ps,
        ):
            # ---------------- persistent tiles ----------------
            w_sb = sg.tile([128, DC * EL], BF16, tag="w")
            wo_sb = sg.tile([128, 16 * TC], BF16, tag="wo")
            cosP = sg.tile([RD, T], BF16, tag="cosP")
            sinP = sg.tile([RD, T], BF16, tag="sinP")
            ident = sg.tile([128, 128], BF16, tag="ident")
            maskM = sg.tile([128, 128], BF16, tag="maskM")
            ones_cb = sg.tile([128, 1], BF16, tag="ones_cb")
            ones_cf = sg.tile([128, 1], F32, tag="ones_cf")
            eps_t = sg.tile([1, 1], F32, tag="eps_t")
            kh = [sg.tile([128, TC], BF16, tag=f"kh{j}", name=f"kh{j}")
                  for j in range(NTC)]
            vh = [sg.tile([128, TC], BF16, tag=f"vh{j}", name=f"vh{j}")
                  for j in range(NTC)]
            vt = [sg.tile([128, TC], BF16, tag=f"vt{j}", name=f"vt{j}")
                  for j in range(NTC)]
            rvns = [sg.tile([1, TC], F32, tag=f"rvns{j}", name=f"rvns{j}")
                    for j in range(NTC)]

            # x tiles live in a rotating 3-chunk window (SBUF pressure);
            # chunk j's DMAs reuse chunk j-3's buffers once qkv(j-3) is done.
            xtl = {}

            def xsl(j, i):
                return xtl[(j, i)]

            def emit_xload(j):
                js = slice(j * TC, (j + 1) * TC)
                for i in range(DC):
                    xtl[(j, i)] = st.tile([128, TC], BF16, tag="xt",
                                          bufs=3 * DC, name=f"x_{j}_{i}")
                    nc.sync.dma_start(
                        out=xtl[(j, i)], in_=xT[i * 128:(i + 1) * 128, js])

            # ---------------- initial DMAs ----------------
            nc.sync.dma_start(out=cosP, in_=csP[0:RD, :])
            nc.sync.dma_start(out=sinP, in_=csP[RD:128, :])
            # interleave k/v weight columns with x chunk 0 so the i-th
            # pass-B matmul can start as soon as its own tiles land
            for i in range(DC):
                nc.sync.dma_start(
                    out=w_sb[:, i * EL + 4 * HD:(i + 1) * EL],
                    in_=wT[i * 128:(i + 1) * 128, 4 * HD:EL],
                )
                xtl[(0, i)] = st.tile([128, TC], BF16, tag="xt",
                                      bufs=3 * DC, name=f"x_0_{i}")
                # scalar engine is an HWDGE too and idle at start: issuing
                # x0 there doubles the initial DMA issue rate
                nc.scalar.dma_start(
                    out=xtl[(0, i)], in_=xT[i * 128:(i + 1) * 128, 0:TC])
            # q weight columns
            for i in range(DC):
                nc.sync.dma_start(
                    out=w_sb[:, i * EL:i * EL + 4 * HD],
                    in_=wT[i * 128:(i + 1) * 128, 0:4 * HD],
                )
            emit_xload(1)
            emit_xload(2)
            # wo tiles: (h, m) at column (h*4+m)*TC
            for h in range(NH_L):
                for m in range(4):
                    nc.sync.dma_start(
                        out=wo_sb[:, (h * 4 + m) * TC:(h * 4 + m + 1) * TC],
                        in_=woT[h * 128:(h + 1) * 128, m * TC:(m + 1) * TC],
                    )
            # chunk 3's x loads wait on chunk 0's buffers; issue them after wo
            # so the in-order sync queue doesn't hold the wo transfers back.
            emit_xload(3)

            make_identity(nc, ident)
            nc.vector.memset(ones_cb, 1.0)
            nc.vector.memset(ones_cf, 1.0)
            nc.vector.memset(eps_t, EPS)
            # lower-triangle 1/0 mask (keep col >= partition)
            nc.gpsimd.memset(maskM, 1.0)
            nc.gpsimd.affine_select(
                out=maskM, in_=maskM,
                compare_op=mybir.AluOpType.is_ge,
                fill=0.0, base=0,
                pattern=[[1, 128]],
                channel_multiplier=-1,
            )

            # warm-up: the PE clock-gate (HAM) needs ~3.4us of sustained
            # activity to lift the 2x throttle; burn dummy transposes on the
            # identity while the initial DMAs land so qkv(0) starts warm
            ps_warm = ps.tile([128, 128], BF16, tag="big", bufs=4,
                              name="ps_warm")
            for _ in range(90):
                nc.tensor.transpose(ps_warm, ident, ident)

            # x^2 tiles for the rms-norm sum (created one chunk ahead)
            x2 = {}
            state = {}

            def emit_x2(j):
                # x^2 tiles plus a running DVE accumulation over the 16
                # contraction blocks (two ping-pong accumulators so the
                # serial dependency never stalls the DVE pipe); qkvB then
                # needs a single ones-matmul for the rms sum instead of 16.
                # Chunk 0 instead keeps per-tile PE matmuls (its x tiles
                # trickle in from the initial DMAs).
                acc = st.tile([128, TC], BF16, tag="x2a", bufs=2,
                              name=f"x2a{j}")
                accB = st.tile([128, TC], BF16, tag="x2b", bufs=2,
                               name=f"x2b{j}")
                for i in range(DC):
                    x2[(j, i)] = st.tile([128, TC], BF16, tag="x2",
                                         bufs=4, name=f"x2_{j}_{i}")
                    nc.vector.tensor_mul(x2[(j, i)], xsl(j, i), xsl(j, i))
                    if j == 0:
                        continue
                    a = acc if i % 2 == 0 else accB
                    if i < 2:
                        nc.vector.tensor_copy(a, x2[(j, i)])
                    else:
                        nc.vector.tensor_add(a, a, x2[(j, i)])
                if j > 0:
                    nc.vector.tensor_add(acc, acc, accB)
                state[("x2a", j)] = acc

            def emit_qkvB(j, rs_mode="first"):
                # ---- pass B: rs sum + k, v on PE ----
                sm_rs = ps.tile([1, TC], F32, tag="big", bufs=4,
                                name=f"smrs{j}")
                ps_k = ps.tile([128, TC], F32, tag="big", bufs=4, name=f"psk{j}")
                ps_v = ps.tile([128, TC], F32, tag="big", bufs=4, name=f"psv{j}")
                if rs_mode == "first":
                    # x2a was accumulated on DVE during attn pair A of the
                    # previous chunk, so this never waits
                    nc.tensor.matmul(sm_rs, ones_cb, state[("x2a", j)],
                                     start=True, stop=True)
                for i in range(DC):
                    wof = i * EL
                    nc.tensor.matmul(
                        ps_k, w_sb[:, wof + 4 * HD:wof + 5 * HD], xsl(j, i),
                        start=(i == 0), stop=(i == DC - 1))
                    nc.tensor.matmul(
                        ps_v, w_sb[:, wof + 5 * HD:wof + 6 * HD], xsl(j, i),
                        start=(i == 0), stop=(i == DC - 1))
                    if rs_mode == "pe":
                        # chunk 0: x tiles trickle in from the initial DMAs,
                        # so reduce per-tile on the PE at DMA pace
                        nc.tensor.matmul(
                            sm_rs, ones_cb, x2[(j, i)],
                            start=(i == 0), stop=(i == DC - 1))
                if rs_mode == "last":
                    nc.tensor.matmul(sm_rs, ones_cb, state[("x2a", j)],
                                     start=True, stop=True)
                # rs = exp(-0.5*ln(mean(x^2)+eps))  (scalar engine only)
                lnr = st.tile([1, TC], F32, tag="lnr", bufs=2)
                nc.scalar.activation(lnr, sm_rs, ACT.Ln, scale=1.0 / D,
                                     bias=eps_t)
                rs_t = st.tile([1, TC], BF16, tag="rs_t", bufs=2)
                nc.scalar.activation(rs_t, lnr, ACT.Exp, scale=-0.5)
                rsb = st.tile([128, TC], BF16, tag="rsb", bufs=2)
                nc.gpsimd.partition_broadcast(rsb, rs_t)
                state[("rsb", j)] = (ps_k, ps_v, rsb)

            def emit_qkvA(j):
                js = slice(j * TC, (j + 1) * TC)
                ps_k, ps_v, rsb = state[("rsb", j)]

                def rope(th):
                    t2s = st.tile([RD, TC], BF16, tag="t2s", bufs=2)
                    nc.vector.stream_shuffle(t2s, th[0:RD], SWAP_MASK)
                    nc.vector.tensor_mul(th[0:RD], th[0:RD], cosP[:, js])
                    nc.vector.tensor_mul(t2s, t2s, sinP[:, js])
                    nc.vector.tensor_add(th[0:RD], th[0:RD], t2s)

                qhj = [
                    st.tile([128, TC], BF16, tag="qh", bufs=8,
                            name=f"qh{j}_{h}")
                    for h in range(NH_L)
                ]
                # two 2-head subpasses keep peak PSUM at 4 accumulator banks
                ps_q01 = [ps.tile([128, TC], F32, tag="big", bufs=4,
                                  name=f"psq{j}_{h}") for h in (0, 1)]
                for i in range(DC):
                    wof = i * EL
                    for h in (0, 1):
                        nc.tensor.matmul(
                            ps_q01[h], w_sb[:, wof + h * HD:wof + (h + 1) * HD],
                            xsl(j, i), start=(i == 0), stop=(i == DC - 1))
                # evacuate with rms scale folded in (DVE); runs during sub2
                nc.vector.tensor_mul(kh[j], ps_k, rsb)
                nc.vector.tensor_mul(vh[j], ps_v, rsb)
                nc.vector.tensor_mul(qhj[0], ps_q01[0], rsb)
                nc.vector.tensor_mul(qhj[1], ps_q01[1], rsb)
                rope(kh[j])
                rope(qhj[0])
                rope(qhj[1])
                ps_q23 = [ps.tile([128, TC], F32, tag="big", bufs=4,
                                  name=f"psq{j}_{h + 2}") for h in (0, 1)]
                for i in range(DC):
                    wof = i * EL
                    for h in (0, 1):
                        nc.tensor.matmul(
                            ps_q23[h],
                            w_sb[:, wof + (h + 2) * HD:wof + (h + 3) * HD],
                            xsl(j, i), start=(i == 0), stop=(i == DC - 1))
                vsq = st.tile([128, TC], BF16, tag="vsq", bufs=2,
                              name=f"vsq{j}")
                nc.vector.tensor_mul(vsq, vh[j], vh[j])
                nc.vector.tensor_mul(qhj[2], ps_q23[0], rsb)
                nc.vector.tensor_mul(qhj[3], ps_q23[1], rsb)
                rope(qhj[2])
                rope(qhj[3])
                state[("vsq", j)] = vsq
                state[("qh", j)] = qhj

            def emit_vt(j):
                # v token-major transpose (PE), own phase so the in-order PE
                # queue never waits on vh here
                ps_vt = ps.tile([128, TC], BF16, tag="big", bufs=4,
                                name=f"psvt{j}")
                for kk in range(TC // 128):
                    nc.tensor.transpose(
                        ps_vt[:, kk * 128:(kk + 1) * 128],
                        vh[j][:, kk * 128:(kk + 1) * 128],
                        ident,
                    )
                nc.vector.tensor_copy(vt[j], ps_vt)

            def emit_vns(j):
                vsq = state[("vsq", j)]
                sm_vns = ps.tile([1, TC], F32, tag="big", bufs=4,
                                 name=f"smvns{j}")
                nc.tensor.matmul(sm_vns, ones_cb, vsq, start=True, stop=True)
                lnv = st.tile([1, TC], F32, tag="lnv", bufs=2)
                nc.scalar.activation(lnv, sm_vns, ACT.Ln, scale=1.0,
                                     bias=eps_t)
                nc.scalar.activation(rvns[j], lnv, ACT.Exp, scale=-1.0)

            def emit_attn_pair(j, pair):
                heads = (0, 1) if pair == 0 else (2, 3)
                qhj = state[("qh", j)]
                nkt = 4 * (j + 1)
                ps_pv = {
                    h: ps.tile([128, TC], F32, tag="big", bufs=4,
                               name=f"pspv{j}_{h}")
                    for h in heads
                }
                sm_sum = ps.tile([64, TC], F32, tag="big", bufs=4,
                                 name=f"smsum{j}_{pair}")
                pT = {}

                def tile_geom(kt):
                    cs = 128 * (kt - 4 * j) if kt >= 4 * j else 0
                    return cs, TC - cs

                def emit_sum_pv(kt):
                    # both tiny-LDW sum mms first, then the two PV mms, so
                    # each PV's 128-col LDWEIGHTS prefetches under the
                    # preceding matmul's stream
                    cs, _w = tile_geom(kt)
                    jk, kk = divmod(kt, 4)
                    pt2 = pT[kt]
                    for idx, h in enumerate(heads):
                        nc.tensor.matmul(
                            sm_sum[32 * (h % 2):32 * (h % 2) + 1, cs:TC],
                            ones_cb, pt2[:, idx * TC + cs:(idx + 1) * TC],
                            start=(kt == 0), stop=(kt == nkt - 1))
                    for idx, h in enumerate(heads):
                        nc.tensor.matmul(
                            ps_pv[h][:, cs:TC],
                            vt[jk][:, kk * 128:(kk + 1) * 128],
                            pt2[:, idx * TC + cs:(idx + 1) * TC],
                            start=(kt == 0), stop=(kt == nkt - 1))

                # both heads' scores land in one double-wide PSUM tile so a
                # single exp covers them (halves the scalar engine's per-tile
                # overhead, which gates attention); sum/pv for tile kt-2
                # issue while kt's scores compute so LDWEIGHTS prefetch is
                # never blocked on a pending semaphore.
                for kt in range(nkt):
                    cs, _w = tile_geom(kt)
                    jk, kk = divmod(kt, 4)
                    ps_sc = ps.tile([128, 2 * TC], F32, tag="sc2", bufs=2,
                                    name=f"pssc{j}_{pair}_{kt}")
                    for idx, h in enumerate(heads):
                        nc.tensor.matmul(
                            ps_sc[:, idx * TC + cs:(idx + 1) * TC],
                            kh[jk][:, kk * 128:(kk + 1) * 128],
                            qhj[h][:, cs:TC],
                            start=True, stop=True)
                    pt2 = st.tile([128, 2 * TC], BF16, tag="pT", bufs=6,
                                  name=f"pt{j}_{pair}_{kt}")
                    # the [TC : TC+cs] span holds stale psum when cs>0; it is
                    # exp'd but never read downstream
                    nc.scalar.activation(
                        pt2[:, cs:2 * TC], ps_sc[:, cs:2 * TC], ACT.Exp,
                        scale=S128)
                    if kt >= 4 * j:
                        # zero strictly-above-diagonal entries in the
                        # 128-wide diagonal block: keep col >= partition.
                        # gpsimd affine_select: on the DVE the wait-on-exp
                        # would serialize the whole attention pipeline.
                        for idx in range(2):
                            nc.gpsimd.affine_select(
                                out=pt2[:, idx * TC + cs:idx * TC + cs + 128],
                                in_=pt2[:, idx * TC + cs:idx * TC + cs + 128],
                                compare_op=mybir.AluOpType.is_ge,
                                fill=0.0, base=0,
                                pattern=[[1, 128]],
                                channel_multiplier=-1,
                            )
                    pT[kt] = pt2
                    if kt > 2:
                        emit_sum_pv(kt - 3)
                for kt in range(max(0, nkt - 3), nkt):
                    emit_sum_pv(kt)
                state[(j, pair)] = (ps_pv, sm_sum)

            def emit_xsa_pre(j, pair):
                heads = (0, 1) if pair == 0 else (2, 3)
                ps_pv, sm_sum = state[(j, pair)]
                pvsb, tu, inv = {}, {}, {}
                for h in heads:
                    pvsb[h] = st.tile([128, TC], BF16, tag="pvsb", bufs=4,
                                      name=f"pvsb{j}_{h}")
                    nc.vector.tensor_copy(pvsb[h], ps_pv[h])
                    lnS = st.tile([1, TC], F32, tag="lnS", bufs=2)
                    nc.scalar.activation(
                        lnS, sm_sum[32 * (h % 2):32 * (h % 2) + 1, :], ACT.Ln)
                    inv[h] = st.tile([1, TC], BF16, tag="inv", bufs=4,
                                     name=f"inv{j}_{h}")
                    nc.scalar.activation(inv[h], lnS, ACT.Exp, scale=-1.0)
                    tu[h] = st.tile([128, TC], BF16, tag="tu", bufs=4,
                                    name=f"tu{j}_{h}")
                    nc.vector.tensor_mul(tu[h], pvsb[h], vh[j])
                state[(j, pair, "pre")] = (pvsb, tu, inv)

            def emit_dots(j, pair):
                heads = (0, 1) if pair == 0 else (2, 3)
                _, tu, _ = state[(j, pair, "pre")]
                sm_dot = ps.tile([64, TC], F32, tag="big", bufs=4,
                                 name=f"smdot{j}_{pair}")
                for h in heads:
                    nc.tensor.matmul(
                        sm_dot[32 * (h % 2):32 * (h % 2) + 1, :],
                        ones_cb, tu[h], start=True, stop=True)
                state[(j, pair, "dot")] = sm_dot

            def emit_xsa_post(j, pair):
                heads = (0, 1) if pair == 0 else (2, 3)
                pvsb, tu, inv = state[(j, pair, "pre")]
                sm_dot = state[(j, pair, "dot")]
                for h in heads:
                    t1 = st.tile([1, TC], F32, tag="t1", bufs=2)
                    nc.vector.tensor_mul(
                        t1, sm_dot[32 * (h % 2):32 * (h % 2) + 1, :], rvns[j])
                    fui = st.tile([1, TC], BF16, tag="fui", bufs=2)
                    nc.vector.tensor_mul(fui, t1, inv[h])
                    invB = st.tile([128, TC], BF16, tag="invB", bufs=2)
                    nc.gpsimd.partition_broadcast(invB, inv[h])
                    fuiB = st.tile([128, TC], BF16, tag="fuiB", bufs=2)
                    nc.gpsimd.partition_broadcast(fuiB, fui)
                    m1 = st.tile([128, TC], BF16, tag="m1", bufs=2)
                    nc.vector.tensor_mul(m1, pvsb[h], invB)
                    m2 = st.tile([128, TC], BF16, tag="m2", bufs=2)
                    nc.vector.tensor_mul(m2, vh[j], fuiB)
                    aot = st.tile([128, TC], BF16, tag="ao", bufs=6,
                                  name=f"ao{j}_{h}")
                    nc.vector.tensor_sub(aot, m1, m2)
                    state[("ao", j, h)] = aot

            def emit_outproj(j, mrange=(0, 1, 2, 3)):
                for m in mrange:
                    ms = slice(m * TC, (m + 1) * TC)
                    for tt in range(4):
                        ps_o = ps.tile([128, TC], F32, tag="big", bufs=4,
                                       name=f"pso{j}_{m}_{tt}")
                        for h in range(NH_L):
                            nc.tensor.matmul(
                                ps_o,
                                state[("ao", j, h)][:, tt * 128:(tt + 1) * 128],
                                wo_sb[:, (h * 4 + m) * TC:(h * 4 + m + 1) * TC],
                                start=(h == 0), stop=(h == NH_L - 1))
                        osb = st.tile([128, TC], F32, tag="osb", bufs=3)
                        # DVE evac: the scalar queue must stay clear for the
                        # next attention phase's exps
                        nc.vector.tensor_copy(osb, ps_o)
                        nc.sync.dma_start(
                            out=outp[(j * 4 + tt) * 128:(j * 4 + tt + 1) * 128,
                                     ms],
                            in_=osb)

            # ================= the schedule =================
            # PE stream: QKV(0) | attnA(0) attnB(0) dotsA(0) | QKV(1) dotsB(0)
            #            outproj(0) | attnA(1) ... so rope(j+1)/XSA(j) on the
            #            other engines always overlap PE matmul phases.
            # PE stream per j: attnA attnB vns dotsA | qkvB(j+1) dotsB
            # qkvA(j+1) | outproj(j) vt(j+1) | attnA(j+1) ... ; the qkv
            # passes cover the XSA/rope latencies on scalar/DVE/gpsimd.
            emit_x2(0)
            emit_qkvB(0, rs_mode="pe")
            emit_qkvA(0)
            emit_vt(0)
            for j in range(NTC):
                emit_attn_pair(j, 0)
                emit_xsa_pre(j, 0)
                if j + 1 < NTC:
                    # x2a chain runs on DVE during attnB(j)'s PE phase
                    emit_x2(j + 1)
                emit_attn_pair(j, 1)
                emit_xsa_pre(j, 1)
                emit_vns(j)
                emit_dots(j, 0)
                emit_xsa_post(j, 0)
                if j == 3:
                    # second half of outproj(2), deferred here so it covers
                    # the XSA-B latency of the final chunk
                    emit_outproj(2, mrange=(2, 3))
                if j + 1 < NTC:
                    # chunk 1's x2a chain races qkvB(1) (chunk 0's attention
                    # window is short), so only defer the rs matmul there
                    emit_qkvB(j + 1, rs_mode="last" if j == 0 else "first")
                emit_dots(j, 1)
                emit_xsa_post(j, 1)
                if j + 1 < NTC:
                    emit_qkvA(j + 1)
                if j == 2:
                    emit_outproj(j, mrange=(0, 1))
                else:
                    emit_outproj(j)
                if j + 1 < NTC:
                    emit_vt(j + 1)

    nc.compile()
    return nc


def _host_inputs(x, cos, sin, w_norm, wq, wk, wv, wo):
    """Build the 8 per-core input maps (host-side layout prep only)."""
    wn = w_norm.astype(np.float32)
    # rope interleave permutation: pair (i, i+32) -> positions (2i, 2i+1)
    p64 = np.empty(64, np.int64)
    p64[0::2] = np.arange(32)
    p64[1::2] = np.arange(32, 64)
    perm = np.concatenate([p64, np.arange(64, HD)])

    cosT = cos.T.astype(np.float32)        # [64, T], cos[i] == cos[i+32]
    sinT = sin.T.astype(np.float32)
    cosP = cosT[p64]
    sinP = np.empty_like(cosP)
    sinP[0::2] = -sinT[:32]
    sinP[1::2] = sinT[:32]
    cs = np.ascontiguousarray(
        np.concatenate([cosP, sinP], axis=0)).astype(ml_dtypes.bfloat16)

    xTs = [np.ascontiguousarray(x[b].T).astype(ml_dtypes.bfloat16)
           for b in range(B)]
    in_maps = []
    for c in range(8):
        b, g = divmod(c, 4)
        wq_s = (wq[g * NH_L * HD:(g + 1) * NH_L * HD] * wn[None, :]).reshape(
            NH_L, HD, D)[:, perm, :].reshape(NH_L * HD, D)
        wk_s = (wk[g * HD:(g + 1) * HD] * wn[None, :])[perm]
        wv_s = wv[g * HD:(g + 1) * HD] * wn[None, :]
        wqkvT = np.ascontiguousarray(
            np.concatenate([wq_s, wk_s, wv_s], axis=0).T
        ).astype(ml_dtypes.bfloat16)                       # [D, 768]
        woT_s = np.ascontiguousarray(
            wo[:, g * NH_L * HD:(g + 1) * NH_L * HD].T
        ).astype(ml_dtypes.bfloat16)                       # [512, D]
        in_maps.append({
            "xT": xTs[b],
            "wqkvT": wqkvT,
            "woT": woT_s,
            "cs": cs,
        })
    return in_maps


def kernel(x, cos, sin, w_norm, wq, wk, wv, wo, rope_dims=64, use_xsa=1,
           **_unused):
    if "nc" not in _CACHE:
        _CACHE["nc"] = _build_nc()
    nc = _CACHE["nc"]
    in_maps = _host_inputs(
        np.asarray(x), np.asarray(cos), np.asarray(sin), np.asarray(w_norm),
        np.asarray(wq), np.asarray(wk), np.asarray(wv), np.asarray(wo),
    )
    res_obj = run_bass_kernel_spmd(nc, in_maps, list(range(8)))
    _CACHE["last"] = res_obj
    res = res_obj.results
    out = np.zeros((B, T, D), dtype=np.float32)
    for c in range(8):
        b = c // 4
        out[b] += np.asarray(res[c]["out"], dtype=np.float32)
    return out
